# revision 20
# baseline (speedup 1.0000x reference)
"""GQA kernel for Trainium2 (Bass/Tile), 8-core head-parallel — v3.

Problem: x(1,2048,1024), Wq(1024,1024)+bq, Wk/Wv(1024,256)+bk/bv,
16 Q heads / 4 KV heads, head_dim 64, full (non-causal) softmax attention.
Reference output is attn(B,H,S,Dh) reshaped DIRECTLY to (B,S,H*Dh):
out rows [h*128,(h+1)*128) belong to head h.

Sharding: core d owns Q heads {2d, 2d+1} (both map to KV head d//2), so each
core computes a contiguous (256,1024) slab of the final output.

Key design points (vs the 148us baseline):
  * Score matmuls for the two heads are row-tiled on the PE array (rows
    0..63 / 64..127 via base partitions) and emitted back-to-back so they
    run CONCURRENTLY, filling the two banks of one (128,1024) score tile.
  * exp() is split across ACT (native Exp) and DVE (Schraudolph exp: one
    tensor_scalar y = s*A + B, f32 PSUM -> int16 whose bits ARE bf16
    exp(s); softmax normalization cancels the systematic scale error).
  * PV matmuls use the 1024-wide [ptA | ptB] rhs (heads share the KV
    head's V) against V' = [V | ones] (ones column -> softmax denominator
    in out row 64), 2x512-col MMs (one fp32 PSUM bank each).
  * QKV biases are folded in as an extra contraction-1 matmul (bias row x
    ones row) so PSUM evacuation is a plain ACT Copy (table-set safe).
  * Projection matmul chains are sprinkled between attention kb steps so
    the PE FIFO never starves the exp engines.
  * Pure-load DMAs alternate the two HWDGE queues (sync/scalar); internal
    copies ride the gpsimd software DGE; output DMAs are emitted last on
    sync. No dependent DMA ever heads-of-line-blocks a load.
"""

import numpy as np

import concourse.bass as bass
import concourse.mybir as mybir
import concourse.tile as tile
from concourse import bacc
from concourse.bass_utils import run_bass_kernel_spmd
from concourse.masks import make_identity

F32 = mybir.dt.float32
BF16 = mybir.dt.bfloat16
I16 = mybir.dt.int16
AF = mybir.ActivationFunctionType
ALU = mybir.AluOpType

S = 2048
DIM = 1024
HD = 64          # head dim
N_CORES = 8
NCH = DIM // 128  # 8 contraction chunks of 128

# Schraudolph exp constants for bf16 bit patterns via int16:
#   bf16_bits(exp(s)) ~= int16(s * 128/ln2 + (127*128 - C))
EXP_A = 128.0 / np.log(2.0)
EXP_C = 7.63
EXP_B = 127.0 * 128.0 - EXP_C

# kb indices (of 16) handled by DVE-Schraudolph; the rest go to ACT.
# Few enough to keep the Schraudolph dither's absmax error ~1.3e-2 (<2e-2
# gate), many enough that ACT (native exp on the rest) stays under the PE
# critical path.
DVE_KB = {3, 7, 11, 13}


def build_kernel():
    nc = bacc.Bacc("TRN2", target_bir_lowering=False, debug=False, num_devices=N_CORES)

    xt_d = nc.dram_tensor("xt", [DIM, S], BF16, kind="ExternalInput").ap()
    wq_d = nc.dram_tensor("wq", [128, NCH, 128], BF16, kind="ExternalInput").ap()
    bq_d = nc.dram_tensor("bq", [1, 128], BF16, kind="ExternalInput").ap()
    wkv_d = nc.dram_tensor("wkv", [128, NCH, 128], BF16, kind="ExternalInput").ap()
    bkv_d = nc.dram_tensor("bkv", [1, 128], BF16, kind="ExternalInput").ap()
    idf_d = nc.dram_tensor("idf", [128, 128], F32, kind="ExternalInput").ap()
    idb_d = nc.dram_tensor("idb", [128, 64], BF16, kind="ExternalInput").ap()
    o_d = nc.dram_tensor("o", [2, S, HD], F32, kind="ExternalOutput").ap()

    with tile.TileContext(nc) as tc:
        with (
            tc.tile_pool(name="const", bufs=1) as const_pool,
            tc.tile_pool(name="persist", bufs=1) as persist_pool,
            tc.tile_pool(name="pt", bufs=4) as pt_pool,
            tc.tile_pool(name="otsb", bufs=2) as otsb_pool,
            tc.tile_pool(name="osb", bufs=2) as osb_pool,
            tc.tile_pool(name="rcp", bufs=3) as rcp_pool,
            tc.tile_pool(name="ps_big", bufs=3, space="PSUM") as ps_big,
            tc.tile_pool(name="ps_ot", bufs=1, space="PSUM") as ps_ot,
        ):
            # single 3-deep PSUM ring (6 banks) shared by score tiles, proj
            # chains and transposes; OT holds the other 2 banks. 3-deep
            # breaks the exp->scores->exp 2-buffer latency chain.
            ps_tr = ps_big
            # ---- persistent SBUF tensors ----
            xT = persist_pool.tile([128, NCH, S], BF16)    # 4 MB
            qt_sb = persist_pool.tile([128, S], BF16)      # heads packed: h*64+d
            kv_sb = persist_pool.tile([128, S], BF16)      # rows 0:64 KT, 64:128 VT
            kt2 = persist_pool.tile([128, S], BF16)        # KT duplicated both halves
            # V' chunks padded to 128 free cols (ones col at 64, zeros above)
            # so the PV LDWEIGHTS qualifies for Fast Weight Load and stops
            # serializing against in-flight matmuls.
            v_sb = persist_pool.tile([128, 16, 128], BF16)

            wq_sb = const_pool.tile([128, NCH, 128], BF16)
            wkv_sb = const_pool.tile([128, NCH, 128], BF16)
            bq_sb = const_pool.tile([1, 128], BF16)
            bkv_sb = const_pool.tile([1, 128], BF16)
            ident = const_pool.tile([128, 128], F32)
            ident2 = const_pool.tile([128, 64], BF16)
            ones_row = const_pool.tile([1, 512], BF16)

            def load_x(xb):
                # 256 KB chunks, alternating the two HWDGE queues
                sl = slice(xb * 512, (xb + 1) * 512)
                for c2 in range(NCH // 2):
                    eng = nc.sync if (xb + c2) % 2 == 0 else nc.scalar
                    eng.dma_start(
                        xT[:, 2 * c2:2 * c2 + 2, sl],
                        xt_d[c2 * 256:(c2 + 1) * 256, sl]
                        .rearrange("(c p) s -> p c s", p=128))

            # load order = need order: x block 0 + kv weights, then the rest
            nc.scalar.dma_start(wkv_sb[:], wkv_d[:])
            load_x(0)
            nc.sync.dma_start(wq_sb[:], wq_d[:])
            nc.sync.dma_start(bq_sb[:], bq_d[:])
            nc.scalar.dma_start(bkv_sb[:], bkv_d[:])
            nc.sync.dma_start(ident[:], idf_d[:])
            nc.scalar.dma_start(ident2[:], idb_d[:])
            nc.vector.memset(ones_row[:], 1.0)
            # V' ones column + zero padding (strided memsets, DVE-local)
            nc.vector.memset(v_sb[:, :, 64:65], 1.0)
            nc.vector.memset(v_sb[:, :, 65:128], 0.0)
            for xb in range(1, 4):
                load_x(xb)

            # ---- PE warm-up during the load window: the HAM clock gate
            # needs ~3.4us of sustained matmul activity to lift the PE from
            # 1.2 to 2.4 GHz, and the first real matmul can't start until
            # x/w DMAs land (~7us). Burn dummy matmuls on a scratch tile so
            # the projections start warm.
            warm_src = const_pool.tile([128, 512], BF16)
            nc.vector.memset(warm_src[:], 0.5)
            for i in range(24):
                wps = ps_ot.tile([128, 512], F32, tag="ot")
                nc.tensor.matmul(wps[:], warm_src[:, 0:128], warm_src[:],
                                 start=True, stop=True)

            class ProjChain:
                """KV or Q projection of one 512-column x block, emitted in
                small matmul chunks so attention MMs interleave in the PE
                FIFO. The bias rides as a final contraction-1 matmul."""

                def __init__(self, xb, which):
                    self.sl = slice(xb * 512, (xb + 1) * 512)
                    self.xb = xb
                    self.which = which
                    self.w = wkv_sb if which == "kv" else wq_sb
                    self.b = bkv_sb if which == "kv" else bq_sb
                    self.ps = ps_tr.tile([128, 512], F32, tag="big")
                    self.c = 0

                def emit(self, n):
                    for _ in range(n):
                        if self.c >= NCH:
                            return
                        nc.tensor.matmul(self.ps[:], self.w[:, self.c, :],
                                         xT[:, self.c, self.sl],
                                         start=(self.c == 0), stop=False)
                        self.c += 1

                def finish(self):
                    self.emit(NCH - self.c)
                    # bias: out[p, q] += b[p] * 1
                    nc.tensor.matmul(self.ps[:], self.b[:], ones_row[:],
                                     start=False, stop=True)
                    dst = kv_sb if self.which == "kv" else qt_sb
                    nc.scalar.activation(dst[:, self.sl], self.ps[:], AF.Copy)
                    if self.which == "kv":
                        # duplicate KT into both kt2 halves (software DGE)
                        nc.gpsimd.dma_start(kt2[0:64, self.sl],
                                            kv_sb[0:64, self.sl])
                        nc.gpsimd.dma_start(kt2[64:128, self.sl],
                                            kv_sb[0:64, self.sl])

            def v_transposes(xb):
                # V' = VT^T chunks (s-part) in bf16
                for j in range(4):
                    kb = xb * 4 + j
                    ps = ps_tr.tile([128, 64], BF16, tag="big")
                    nc.tensor.matmul(
                        ps[:], kv_sb[64:128, kb * 128:(kb + 1) * 128],
                        ident2[64:128, :], is_transpose=True)
                    nc.vector.tensor_copy(v_sb[:, kb, 0:64], ps[:])

            def attn_kb(qb, kb, OT):
                """One kb step of q-block qb: scores pair, exp, PV pair."""
                qsl = slice(qb * 512, (qb + 1) * 512)
                ksl = slice(kb * 128, (kb + 1) * 128)
                pss = ps_big.tile([128, 1024], F32, tag="big")
                # two heads row-tiled: rows 0..63 / 64..127, separate banks
                nc.tensor.matmul(pss[:, 0:512], kt2[0:64, ksl],
                                 qt_sb[0:64, qsl], start=True, stop=True)
                nc.tensor.matmul(pss[:, 512:1024], kt2[64:128, ksl],
                                 qt_sb[64:128, qsl], start=True, stop=True)
                if kb in DVE_KB:
                    pt = pt_pool.tile([128, 1024], I16, tag="ptD")
                    nc.vector.tensor_scalar(pt[:], pss[:], EXP_A, EXP_B,
                                            ALU.mult, ALU.add)
                    rhs = pt[:].bitcast(BF16)
                else:
                    pt = pt_pool.tile([128, 1024], BF16, tag="ptA")
                    nc.scalar.activation(pt[:], pss[:], AF.Exp)
                    rhs = pt[:]
                for u in range(2):
                    nc.tensor.matmul(
                        OT[:, u * 512:(u + 1) * 512],
                        v_sb[:, kb, :],
                        rhs[:, u * 512:(u + 1) * 512],
                        start=(kb == 0), stop=(kb == 15),
                        skip_group_check=True)

            def evac_qb(OT):
                ot_sb = otsb_pool.tile([65, 1024], F32, tag="ot_sb")
                nc.vector.tensor_copy(ot_sb[:], OT[0:65, :])
                return ot_sb

            def norm_qb(qb, ot_sb):
                """Transpose to s-major, normalize, DMA out (sync queue —
                emitted after all loads so no head-of-line blocking)."""
                qsl = slice(qb * 512, (qb + 1) * 512)
                o_sb = osb_pool.tile([128, 8, HD], F32, tag="o_sb")
                for j in range(8):
                    ps = ps_tr.tile([128, 65], F32, tag="big")
                    nc.tensor.transpose(
                        ps[:], ot_sb[:, j * 128:(j + 1) * 128], ident[:65, :65])
                    rcp = rcp_pool.tile([128, 1], F32, tag="rcp")
                    nc.vector.reciprocal(rcp[:], ps[:, 64:65])
                    nc.vector.tensor_scalar_mul(o_sb[:, j, :], ps[:, 0:64], rcp[:])
                for h in range(2):
                    nc.sync.dma_start(
                        o_d[h, qsl, :].rearrange("(t j) c -> j t c", j=128),
                        o_sb[:, h * 4:(h + 1) * 4, :])

            # ---- emission ----
            # only block 0 projected before attention; blocks 1..3 sprinkle
            # into q-block 0's kb loop (each K block kb needs x block kb//4).
            for which in ("kv", "q"):
                ProjChain(0, which).finish()
            v_transposes(0)

            chains = {}

            def do_chain(xb, which, n, fin=False):
                ch = chains.get((xb, which))
                if ch is None:
                    ch = chains[xb, which] = ProjChain(xb, which)
                ch.emit(n)
                if fin:
                    ch.finish()

            sprinkle = {
                0: lambda: do_chain(1, "kv", 8, True),
                1: lambda: do_chain(1, "q", 8, True),
                2: lambda: v_transposes(1),
                4: lambda: do_chain(2, "kv", 4),
                5: lambda: do_chain(2, "kv", 4, True),
                6: lambda: do_chain(2, "q", 4),
                7: lambda: (do_chain(2, "q", 4, True), v_transposes(2)),
                9: lambda: do_chain(3, "kv", 4),
                10: lambda: do_chain(3, "kv", 4, True),
                11: lambda: (do_chain(3, "q", 4, True), v_transposes(3)),
            }
            OT = ps_ot.tile([128, 1024], F32, tag="ot")
            for kb in range(16):
                attn_kb(0, kb, OT)
                if kb in sprinkle:
                    sprinkle[kb]()
            ot_prev, qb_prev = evac_qb(OT), 0

            for qb in range(1, 4):
                OT = ps_ot.tile([128, 1024], F32, tag="ot")
                for kb in range(6):
                    attn_kb(qb, kb, OT)
                norm_qb(qb_prev, ot_prev)
                for kb in range(6, 16):
                    attn_kb(qb, kb, OT)
                ot_prev, qb_prev = evac_qb(OT), qb
            norm_qb(qb_prev, ot_prev)

    nc.compile()
    return nc


_NC_CACHE = None


def make_in_maps(inputs):
    import ml_dtypes
    x = np.asarray(inputs["x"], np.float32).reshape(S, DIM)
    xt = np.ascontiguousarray(x.T).astype(ml_dtypes.bfloat16)
    Wq = np.asarray(inputs["Wq"], np.float32)
    bq = np.asarray(inputs["bq"], np.float32)
    Wk = np.asarray(inputs["Wk"], np.float32)
    bk = np.asarray(inputs["bk"], np.float32)
    Wv = np.asarray(inputs["Wv"], np.float32)
    bv = np.asarray(inputs["bv"], np.float32)

    def chunked(w):
        # (1024, 128) -> [128 part, chunk, 128] bf16
        return np.ascontiguousarray(
            w.reshape(NCH, 128, 128).transpose(1, 0, 2)
        ).astype(ml_dtypes.bfloat16)

    in_maps = []
    for d in range(N_CORES):
        g = d // 2
        wkv = np.concatenate(
            [Wk[:, g * 64:(g + 1) * 64], Wv[:, g * 64:(g + 1) * 64]], axis=1)
        bkv = np.concatenate([bk[g * 64:(g + 1) * 64], bv[g * 64:(g + 1) * 64]])
        idf = np.eye(128, dtype=np.float32)
        idb = np.concatenate([np.eye(64), np.eye(64)]).astype(ml_dtypes.bfloat16)
        in_maps.append({
            "xt": xt,
            "wq": chunked(Wq[:, d * 128:(d + 1) * 128] / 8.0),
            "bq": (bq[d * 128:(d + 1) * 128] / 8.0).reshape(1, 128)
                  .astype(ml_dtypes.bfloat16),
            "wkv": chunked(wkv),
            "bkv": bkv.reshape(1, 128).astype(ml_dtypes.bfloat16),
            "idf": idf,
            "idb": np.ascontiguousarray(idb),
        })
    return in_maps


def kernel(**inputs) -> np.ndarray:
    global _NC_CACHE
    if _NC_CACHE is None:
        _NC_CACHE = build_kernel()
    nc = _NC_CACHE
    in_maps = make_in_maps(inputs)
    res = run_bass_kernel_spmd(nc, in_maps, list(range(N_CORES)))
    blocks = [np.asarray(res.results[d]["o"]).reshape(256, DIM) for d in range(N_CORES)]
    return np.concatenate(blocks, axis=0).reshape(1, S, DIM).astype(np.float32)
